# revision 29
# baseline (speedup 1.0000x reference)
"""Trainium2 Bass kernel for the Tolles-Lawson custom loss.

reference:
    c = model_output[:, :18]; d = model_output[:, 18:19]
    tmp = sum(A * (beta_TL + c), axis=1, keepdims=True) + d
    L = mean((tmp - y)^2) + mean((tmp - B_tl)^2)

Sharding: pure data parallel over rows on 8 cores. Each core gets
R = 501,760 rows (core 7 zero-padded; zero rows contribute 0 to both
sums). Per-core partial sums [128, 2*NT] are summed on the host and
divided by N (the all-reduce of the two MSE sums from the sharding
hint, done host-side since the output is tiny).

default mode "bf16c" (one-stream, f32-staged, bf16 on-chip compute):
  * Host interleaves every input into ONE f32 stream of 40-value rows
    [c(18) | d | 0pad | A(18) | y | b] tiled [NT=7, 128, 560*40], so
    each tile is a single 11.5 MB DMA (89.6 KB contiguous per
    partition) that runs near the HBM roofline; the f32 -> bf16 cast
    happens inside that SWDGE DMA, so HBM still streams the full f32
    bytes but SBUF + DVE see bf16.
  * The 20-wide [c|d|0] block keeps bf16 rows even-length and
    4B-aligned, which lets every big DVE op run in 2x perf mode; the
    row reduce runs over all 20 columns so d rides along for free
    (the pad column contributes 0).
  * DVE: bc += [beta,0,0] (2x, beta read via a stride-0 broadcast AP),
    bc[:, :, :18] *= A (2x, in place), tmp = reduce_x(bc) (2x),
    e1/e2 = tmp - y/b. ACT: Square+accum into per-tile partial-sum
    columns. Per-core DVE ~130us, ACT ~10us, both hidden under the
    ~200us DMA stream; io pool is 4-deep to keep the DMA queue full.
  * Numerics: bf16 rounding of c/A/y/b is row-random and washes out in
    the 4M-row mean; bf16 beta is a systematic ~5e-4 relative bias on
    the loss - well inside the 2e-2 gate (measured total 6e-4).

Older modes kept for ablation: f32_dma_accum (previous default: f32
DVE pipeline, beta-add via CCE accumulate DMAs, ~300us), bf16p
(4-stream SWDGE-cast variant, ~245us), pure_dma (loads-only probe),
bf16s/bf16cs (stage bf16 in DRAM: halves HBM bytes, ~170us, NOT used
because staged inputs must preserve input dtypes).
"""

import numpy as np
import ml_dtypes

import concourse.bacc as bacc
import concourse.mybir as mybir
from concourse import tile
from concourse.bass_utils import run_bass_kernel_spmd

N_TOTAL = 4_000_000
NCOEF = 18
C = NCOEF + 1  # 19: coeffs + bias column
P = 128
T = 245          # rows per partition per tile
NT = 16          # tiles per core
RP = T * NT      # 3920 rows per partition
R = P * RP       # 501,760 rows per core
N_CORES = 8

f32 = mybir.dt.float32
bf16 = mybir.dt.bfloat16

MODE = "bf16c"  # bf16 | f32_dma_accum | f32_dve | bf16p | bf16s | bf16c | bf16cs

# bf16p mode: pad model_output to 20 columns (even row length, 4B-aligned
# bf16 rows) so every DVE op runs in 2x perf mode; SWDGE cast-DMA converts
# all streams f32->bf16 on the way into SBUF; ACT does square+accumulate.
T2 = 560   # rows per partition per tile
NT2 = 7    # tiles per core (T2*NT2 == 3920 rows/partition, same R)
C2 = 20    # 18 coeffs + bias + zero pad

_cached = {}


def _build_nc_v2(rep=1, T=T2, NT=NT2, yb_hwdge=False, probe_dma=False,
                 staged=False):
    key = ("nc2", rep, T, NT, yb_hwdge, probe_dma, staged)
    if key in _cached:
        return _cached[key]
    dt_ext = bf16 if staged else f32
    nc = bacc.Bacc(None)
    mo_ext = nc.declare_dram_parameter("mo", [NT, P, T * C2], dt_ext, isOutput=False)
    a_ext = nc.declare_dram_parameter("a", [NT, P, T * NCOEF], dt_ext, isOutput=False)
    y_ext = nc.declare_dram_parameter("y", [NT, P, T], dt_ext, isOutput=False)
    b_ext = nc.declare_dram_parameter("b", [NT, P, T], dt_ext, isOutput=False)
    vb_ext = nc.declare_dram_parameter("vb", [P, T * C2], bf16, isOutput=False)
    out_ext = nc.declare_dram_parameter("out", [P, 2 * NT], f32, isOutput=True)
    dma_in = nc.sync.dma_start if staged else nc.gpsimd.dma_start

    add = mybir.AluOpType.add
    sub = mybir.AluOpType.subtract
    mult = mybir.AluOpType.mult
    AX = mybir.AxisListType.X
    sq = mybir.ActivationFunctionType.Square

    with tile.TileContext(nc) as tc:
        with tc.tile_pool(name="consts", bufs=1) as consts, \
             tc.tile_pool(name="io", bufs=3) as io, \
             tc.tile_pool(name="work", bufs=3) as work, \
             tc.tile_pool(name="accp", bufs=1) as accp:
            vb = consts.tile([P, T * C2], bf16)
            nc.sync.dma_start(out=vb[:], in_=vb_ext[:])
            accs = accp.tile([P, 2 * NT], f32)
            if probe_dma:
                nc.vector.memset(accs[:], 0.0)
            for r in range(rep):
                for i in range(NT):
                    bc = io.tile([P, T * C2], bf16, tag="bc")
                    dma_in(out=bc[:], in_=mo_ext[i])
                    a_t = io.tile([P, T * NCOEF], bf16, tag="a")
                    dma_in(out=a_t[:], in_=a_ext[i])
                    if yb_hwdge:
                        y_t = io.tile([P, T], f32, tag="y")
                        nc.sync.dma_start(out=y_t[:], in_=y_ext[i])
                        b_t = io.tile([P, T], f32, tag="b")
                        nc.sync.dma_start(out=b_t[:], in_=b_ext[i])
                    else:
                        y_t = io.tile([P, T], bf16, tag="y")
                        dma_in(out=y_t[:], in_=y_ext[i])
                        b_t = io.tile([P, T], bf16, tag="b")
                        dma_in(out=b_t[:], in_=b_ext[i])
                    if probe_dma:
                        # consume each tile so DMAs aren't dead code
                        nc.vector.tensor_tensor(
                            accs[:, 2 * i:2 * i + 1], bc[:, 0:1], a_t[:, 0:1], add)
                        nc.vector.tensor_tensor(
                            accs[:, 2 * i + 1:2 * i + 2], y_t[:, 0:1], b_t[:, 0:1], add)
                        continue
                    nc.vector.tensor_tensor(bc[:], bc[:], vb[:], add)
                    bc3 = bc[:].rearrange("p (t c) -> p t c", c=C2)
                    a3 = a_t[:].rearrange("p (t c) -> p t c", c=NCOEF)
                    nc.vector.tensor_tensor(
                        bc3[:, :, 0:NCOEF], a3, bc3[:, :, 0:NCOEF], mult)
                    tmp = work.tile([P, T], bf16, tag="tmp")
                    with nc.allow_low_precision(reason="bf16 row sums; error washes out over 4M rows"):
                        nc.vector.tensor_reduce(tmp[:], bc3, axis=AX, op=add)
                    e1 = work.tile([P, T], bf16, tag="e1")
                    nc.vector.tensor_tensor(e1[:], tmp[:], y_t[:], sub)
                    s1 = work.tile([P, T], bf16, tag="s1")
                    nc.scalar.activation(s1[:], e1[:], sq,
                                         accum_out=accs[:, 2 * i:2 * i + 1])
                    nc.vector.tensor_tensor(tmp[:], tmp[:], b_t[:], sub)
                    s2 = work.tile([P, T], bf16, tag="s2")
                    nc.scalar.activation(s2[:], tmp[:], sq,
                                         accum_out=accs[:, 2 * i + 1:2 * i + 2])
            nc.sync.dma_start(out=out_ext[:], in_=accs[:])
    nc.finalize()
    _cached[key] = nc
    return nc


CW = 40  # combined row width: [c(18) d pad | A(18) | y | b]


def _build_nc_c(rep=1, T=T2, NT=NT2, bufs=3, staged=False, split=1,
                vb_bcast=False, probe_dma=False):
    """One interleaved stream per tile: a single SWDGE cast-DMA loads
    [c|d|0|A|y|b] rows (f32 in DRAM, bf16 in SBUF). staged=True keeps
    bf16 in DRAM and uses a plain HWDGE load instead. split>1 issues
    the tile load as that many equal contiguous DMAs. vb_bcast reads
    beta through a stride-0 broadcast AP instead of a replicated tile."""
    key = ("ncc", rep, T, NT, bufs, staged, split, vb_bcast, probe_dma)
    if key in _cached:
        return _cached[key]
    nc = bacc.Bacc(None)
    big_ext = nc.declare_dram_parameter("big", [NT, P, T * CW],
                                        bf16 if staged else f32, isOutput=False)
    vb_ext = nc.declare_dram_parameter(
        "vb", [P, C2] if vb_bcast else [P, T * C2], bf16, isOutput=False)
    out_ext = nc.declare_dram_parameter("out", [P, 2 * NT], f32, isOutput=True)
    dma_in = nc.sync.dma_start if staged else nc.gpsimd.dma_start

    add = mybir.AluOpType.add
    sub = mybir.AluOpType.subtract
    mult = mybir.AluOpType.mult
    AX = mybir.AxisListType.X
    sq = mybir.ActivationFunctionType.Square

    with tile.TileContext(nc) as tc:
        with tc.tile_pool(name="consts", bufs=1) as consts, \
             tc.tile_pool(name="io", bufs=bufs) as io, \
             tc.tile_pool(name="work", bufs=3) as work, \
             tc.tile_pool(name="accp", bufs=1) as accp:
            vb = consts.tile([P, C2] if vb_bcast else [P, T * C2], bf16)
            nc.sync.dma_start(out=vb[:], in_=vb_ext[:])
            accs = accp.tile([P, 2 * NT], f32)
            if probe_dma:
                nc.vector.memset(accs[:], 0.0)
            for r in range(rep):
                for i in range(NT):
                    big = io.tile([P, T * CW], bf16, tag="big")
                    if split == 1:
                        dma_in(out=big[:], in_=big_ext[i])
                    else:
                        w = T * CW
                        step = w // split
                        for s0 in range(0, w, step):
                            dma_in(out=big[:, s0:s0 + step],
                                   in_=big_ext[i][:, s0:s0 + step])
                    if probe_dma:
                        # loads-only roofline probe: consume one element
                        nc.vector.tensor_tensor(
                            accs[:, 2 * i:2 * i + 1], big[:, 0:1],
                            big[:, 1:2], mybir.AluOpType.add)
                        continue
                    big3 = big[:].rearrange("p (t w) -> p t w", w=CW)
                    bc3 = big3[:, :, 0:C2]
                    a3 = big3[:, :, C2:38]
                    y_t = big3[:, :, 38]
                    b_t = big3[:, :, 39]
                    if vb_bcast:
                        vb3 = vb[:].unsqueeze(1).broadcast_to([P, T, C2])
                    else:
                        vb3 = vb[:].rearrange("p (t c) -> p t c", c=C2)
                    nc.vector.tensor_tensor(bc3, bc3, vb3, add)
                    nc.vector.tensor_tensor(
                        bc3[:, :, 0:NCOEF], a3, bc3[:, :, 0:NCOEF], mult)
                    tmp = work.tile([P, T], bf16, tag="tmp")
                    with nc.allow_low_precision(reason="bf16 row sums; error washes out over 4M rows"):
                        nc.vector.tensor_reduce(tmp[:], bc3, axis=AX, op=add)
                    e1 = work.tile([P, T], bf16, tag="e1")
                    nc.vector.tensor_tensor(e1[:], tmp[:], y_t, sub)
                    s1 = work.tile([P, T], bf16, tag="s1")
                    nc.scalar.activation(s1[:], e1[:], sq,
                                         accum_out=accs[:, 2 * i:2 * i + 1])
                    nc.vector.tensor_tensor(tmp[:], tmp[:], b_t, sub)
                    s2 = work.tile([P, T], bf16, tag="s2")
                    nc.scalar.activation(s2[:], tmp[:], sq,
                                         accum_out=accs[:, 2 * i + 1:2 * i + 2])
            nc.sync.dma_start(out=out_ext[:], in_=accs[:])
    nc.finalize()
    _cached[key] = nc
    return nc


def _prepare_c(model_output, y, A, B_tl, beta_TL, T=T2, NT=NT2, staged=False,
               vb_bcast=False):
    model_output = np.asarray(model_output, dtype=np.float32)
    y = np.asarray(y, dtype=np.float32)
    A = np.asarray(A, dtype=np.float32)
    B_tl = np.asarray(B_tl, dtype=np.float32)
    beta_TL = np.asarray(beta_TL, dtype=np.float32)

    vrow = np.concatenate([beta_TL, np.zeros(2, np.float32)])  # [20]
    vb = np.tile(vrow.astype(ml_dtypes.bfloat16), (P, T if not vb_bcast else 1))

    dt_np = ml_dtypes.bfloat16 if staged else np.float32
    big_s = []
    for i in range(N_CORES):
        s = np.zeros((R, CW), dt_np)
        lo = i * R
        hi = min(lo + R, N_TOTAL)
        n = hi - lo
        s[:n, 0:C] = model_output[lo:hi]
        s[:n, C2:C2 + NCOEF] = A[lo:hi]
        s[:n, 38] = y[lo:hi, 0]
        s[:n, 39] = B_tl[lo:hi, 0]
        big_s.append(s.reshape(NT, P, T * CW))
    return [{"big": big_s[i], "vb": vb} for i in range(N_CORES)]


def _prepare_v2(model_output, y, A, B_tl, beta_TL, T=T2, NT=NT2, staged=False):
    model_output = np.asarray(model_output, dtype=np.float32)
    y = np.asarray(y, dtype=np.float32)
    A = np.asarray(A, dtype=np.float32)
    B_tl = np.asarray(B_tl, dtype=np.float32)
    beta_TL = np.asarray(beta_TL, dtype=np.float32)

    vrow = np.concatenate([beta_TL, np.zeros(2, np.float32)])  # [20]
    vb = np.tile(vrow.astype(ml_dtypes.bfloat16), (P, T))  # [128, T*20]

    dt_np = ml_dtypes.bfloat16 if staged else np.float32
    mo_s = []
    for i in range(N_CORES):
        s = np.zeros((R, C2), dt_np)
        lo = i * R
        hi = min(lo + R, N_TOTAL)
        s[: hi - lo, :C] = model_output[lo:hi]
        mo_s.append(s.reshape(NT, P, T * C2))
    a_s = [x.reshape(NT, P, T * NCOEF).astype(dt_np) for x in _shard(A, NCOEF)]
    y_s = [x.reshape(NT, P, T).astype(dt_np) for x in _shard(y, 1)]
    b_s = [x.reshape(NT, P, T).astype(dt_np) for x in _shard(B_tl, 1)]
    return [
        {"mo": mo_s[i], "a": a_s[i], "y": y_s[i], "b": b_s[i], "vb": vb}
        for i in range(N_CORES)
    ]


def _build_nc(T=T, NT=NT, mode=None, rep=1, W1=760):
    mode = mode or MODE
    if mode == "bf16c":
        return _build_nc_c(rep=rep, bufs=4, vb_bcast=True)
    if mode in ("bf16p", "bf16s"):
        return _build_nc_v2(rep=rep, staged=(mode == "bf16s"))
    W1 = min(W1, T * C // 2 // 8 * 8)  # clamp for small test configs
    key = ("nc", T, NT, mode, rep, W1)
    if key in _cached:
        return _cached[key]
    use_bf16 = mode == "bf16"
    dt_in = bf16 if use_bf16 else f32
    nc = bacc.Bacc(None)
    mo_ext = nc.declare_dram_parameter("mo", [NT, P, T * C], f32, isOutput=False)
    a_ext = nc.declare_dram_parameter("a", [NT, P, T * NCOEF], f32, isOutput=False)
    if mode == "f32_split":
        yb_ext = nc.declare_dram_parameter("yb", [NT, P, T * 2], f32, isOutput=False)
    else:
        y_ext = nc.declare_dram_parameter("y", [NT, P, T], f32, isOutput=False)
        b_ext = nc.declare_dram_parameter("b", [NT, P, T], f32, isOutput=False)
    vb_ext = nc.declare_dram_parameter("vb", [P, T * C], dt_in, isOutput=False)
    out_ext = nc.declare_dram_parameter("out", [P, 2 * NT], f32, isOutput=True)

    add = mybir.AluOpType.add
    sub = mybir.AluOpType.subtract
    mult = mybir.AluOpType.mult
    AX = mybir.AxisListType.X

    with tile.TileContext(nc) as tc:
        with tc.tile_pool(name="consts", bufs=1) as consts, \
             tc.tile_pool(name="io", bufs=3) as io, \
             tc.tile_pool(name="work", bufs=3) as work, \
             tc.tile_pool(name="accp", bufs=1) as accp:
            vb = consts.tile([P, T * C], dt_in)
            nc.sync.dma_start(out=vb[:], in_=vb_ext[:])
            accs = accp.tile([P, 2 * NT], f32)

            if use_bf16:
                # two persistent [A | 1.0] buffers; col 18 preset to 1.0,
                # ACT rewrites cols 0:18 each tile
                a19s = [consts.tile([P, T * C], dt_in, tag=f"a19_{j}",
                                    name=f"a19_{j}") for j in range(2)]
                for j in range(2):
                    nc.vector.memset(a19s[j][:], 1.0)

            if mode == "f32_split":
                nc.vector.memset(accs[:], 0.0)
                sq = mybir.ActivationFunctionType.Square
                for r in range(rep):
                    for i in range(NT):
                        a_t = io.tile([P, T * NCOEF], f32, tag="a")
                        nc.sync.dma_start(out=a_t[:], in_=a_ext[i])
                        yb_t = io.tile([P, T * 2], f32, tag="yb")
                        nc.sync.dma_start(out=yb_t[:], in_=yb_ext[i])

                        # beta+c add, split: cols [0:W1) accumulate during
                        # the DMA (CCE, needs ACT prefill + <=2048-elem
                        # descriptors); cols [W1:) plain load + DVE add
                        bc = work.tile([P, T * C], f32, tag="bc")
                        nc.scalar.copy(out=bc[:, 0:W1], in_=vb[:, 0:W1])
                        for s0 in range(0, W1, 2048):
                            s1 = min(s0 + 2048, W1)
                            nc.gpsimd.dma_start(out=bc[:, s0:s1],
                                                in_=mo_ext[i][:, s0:s1],
                                                accum_op=add)
                        nc.sync.dma_start(out=bc[:, W1:], in_=mo_ext[i][:, W1:])
                        nc.vector.tensor_tensor(bc[:, W1:], bc[:, W1:],
                                                vb[:, W1:], add)

                        bc3 = bc[:].rearrange("p (t c) -> p t c", c=C)
                        prod = work.tile([P, T * NCOEF], f32, tag="prod")
                        nc.vector.tensor_tensor(
                            prod[:], a_t[:], bc3[:, :, 0:NCOEF], mult)
                        tmp = work.tile([P, T], f32, tag="tmp")
                        nc.vector.tensor_reduce(
                            tmp[:], prod[:].rearrange("p (t c) -> p t c", c=NCOEF),
                            axis=AX, op=add)
                        nc.vector.tensor_tensor(tmp[:], tmp[:], bc3[:, :, NCOEF], add)

                        yb3 = yb_t[:].rearrange("p (t c) -> p t c", c=2)
                        e1 = work.tile([P, T], f32, tag="e1")
                        nc.vector.tensor_tensor(e1[:], tmp[:], yb3[:, :, 0], sub)
                        sq1 = work.tile([P, T], f32, tag="sq1")
                        nc.scalar.activation(sq1[:], e1[:], sq,
                                             accum_out=accs[:, 2 * i:2 * i + 1])
                        e2 = work.tile([P, T], f32, tag="e2")
                        nc.vector.tensor_tensor(e2[:], tmp[:], yb3[:, :, 1], sub)
                        sq2 = work.tile([P, T], f32, tag="sq2")
                        nc.scalar.activation(sq2[:], e2[:], sq,
                                             accum_out=accs[:, 2 * i + 1:2 * i + 2])

            for r in range(rep if mode != "f32_split" else 0):
                for i in range(NT):
                    if mode == "pure_dma":
                        # bandwidth-roofline probe: loads only, plus a tiny
                        # consume of each tile so nothing is dead code
                        bc = work.tile([P, T * C], f32, tag="bc")
                        nc.sync.dma_start(out=bc[:], in_=mo_ext[i])
                        a_t = io.tile([P, T * NCOEF], f32, tag="a")
                        nc.sync.dma_start(out=a_t[:], in_=a_ext[i])
                        y_t = io.tile([P, T], f32, tag="y")
                        nc.sync.dma_start(out=y_t[:], in_=y_ext[i])
                        b_t = io.tile([P, T], f32, tag="b")
                        nc.sync.dma_start(out=b_t[:], in_=b_ext[i])
                        nc.vector.tensor_tensor(
                            accs[:, 2 * i:2 * i + 1], bc[:, 0:1], a_t[:, 0:1], add)
                        nc.vector.tensor_tensor(
                            accs[:, 2 * i + 1:2 * i + 2], y_t[:, 0:1], b_t[:, 0:1], add)
                        continue
                    a_t = io.tile([P, T * NCOEF], dt_in, tag="a")
                    if use_bf16:
                        nc.gpsimd.dma_start(out=a_t[:], in_=a_ext[i])
                    else:
                        nc.sync.dma_start(out=a_t[:], in_=a_ext[i])
                    y_t = io.tile([P, T], f32, tag="y")
                    nc.sync.dma_start(out=y_t[:], in_=y_ext[i])
                    b_t = io.tile([P, T], f32, tag="b")
                    nc.sync.dma_start(out=b_t[:], in_=b_ext[i])

                    bc = work.tile([P, T * C], dt_in, tag="bc")
                    if use_bf16:
                        mo_t = io.tile([P, T * C], dt_in, tag="mo")
                        nc.gpsimd.dma_start(out=mo_t[:], in_=mo_ext[i])
                        nc.vector.tensor_tensor(bc[:], mo_t[:], vb[:], add)
                    elif mode == "f32_dma_accum":
                        nc.scalar.copy(out=bc[:], in_=vb[:])
                        # CCE (inline DMA accumulate ALU) handles at most
                        # 2048 elements per descriptor; larger runs are
                        # silently wrong, and max_dma_last_dim is not
                        # honored for Tile's symbolic APs — slice manually
                        w = T * C
                        nslice = -(-w // 2048)
                        step = -(-w // nslice)
                        step += (-step) % 8  # keep 32B-aligned slice starts
                        for s0 in range(0, w, step):
                            s1 = min(s0 + step, w)
                            nc.gpsimd.dma_start(out=bc[:, s0:s1],
                                                in_=mo_ext[i][:, s0:s1],
                                                accum_op=add)
                    else:
                        nc.sync.dma_start(out=bc[:], in_=mo_ext[i])
                        eng = nc.vector if mode == "f32_dve" else nc.gpsimd
                        eng.tensor_tensor(bc[:], bc[:], vb[:], add)

                    bc3 = bc[:].rearrange("p (t c) -> p t c", c=C)
                    tmp = work.tile([P, T], f32, tag="tmp")
                    if use_bf16:
                        a19 = a19s[i % 2]
                        nc.scalar.copy(
                            out=a19[:].rearrange("p (t c) -> p t c", c=C)[:, :, 0:NCOEF],
                            in_=a_t[:].rearrange("p (t c) -> p t c", c=NCOEF),
                        )
                        nc.vector.tensor_tensor(bc[:], a19[:], bc[:], mult)
                        nc.vector.tensor_reduce(tmp[:], bc3, axis=AX, op=add)
                    else:
                        prod = work.tile([P, T * NCOEF], dt_in, tag="prod")
                        nc.vector.tensor_tensor(
                            prod[:], a_t[:], bc3[:, :, 0:NCOEF], mult)
                        nc.vector.tensor_reduce(
                            tmp[:], prod[:].rearrange("p (t c) -> p t c", c=NCOEF),
                            axis=AX, op=add)
                        nc.vector.tensor_tensor(tmp[:], tmp[:], bc3[:, :, NCOEF], add)

                    e1 = work.tile([P, T], f32, tag="e1")
                    nc.vector.tensor_tensor(e1[:], tmp[:], y_t[:], sub)
                    nc.vector.tensor_tensor(e1[:], e1[:], e1[:], mult)
                    nc.vector.tensor_reduce(
                        accs[:, 2 * i:2 * i + 1], e1[:], axis=AX, op=add)
                    e2 = work.tile([P, T], f32, tag="e2")
                    nc.vector.tensor_tensor(e2[:], tmp[:], b_t[:], sub)
                    nc.vector.tensor_tensor(e2[:], e2[:], e2[:], mult)
                    nc.vector.tensor_reduce(
                        accs[:, 2 * i + 1:2 * i + 2], e2[:], axis=AX, op=add)

            nc.sync.dma_start(out=out_ext[:], in_=accs[:])
    nc.finalize()
    _cached[key] = nc
    return nc


def _shard(arr, ncols):
    """Split [N_TOTAL, ncols] f32 into 8 shards of [NT, P, T*ncols]."""
    shards = []
    for i in range(N_CORES):
        lo = i * R
        hi = lo + R
        if hi <= N_TOTAL:
            s = arr[lo:hi]
        else:
            s = np.zeros((R, ncols), dtype=arr.dtype)
            s[: N_TOTAL - lo] = arr[lo:]
        shards.append(np.ascontiguousarray(s).reshape(NT, P, T * ncols))
    return shards


def _prepare_in_maps(model_output, y, A, B_tl, beta_TL, mode=None):
    mode = mode or MODE
    if mode == "bf16c":
        return _prepare_c(model_output, y, A, B_tl, beta_TL, vb_bcast=True)
    if mode in ("bf16p", "bf16s"):
        return _prepare_v2(model_output, y, A, B_tl, beta_TL,
                           staged=(mode == "bf16s"))
    model_output = np.asarray(model_output, dtype=np.float32)
    y = np.asarray(y, dtype=np.float32)
    A = np.asarray(A, dtype=np.float32)
    B_tl = np.asarray(B_tl, dtype=np.float32)
    beta_TL = np.asarray(beta_TL, dtype=np.float32)

    vrow = np.concatenate([beta_TL, np.zeros(1, np.float32)])  # [19]
    dt_np = ml_dtypes.bfloat16 if mode == "bf16" else np.float32
    vb = np.tile(vrow.astype(dt_np), (P, T))  # [128, T*19]

    mo_s = _shard(model_output, C)
    a_s = _shard(A, NCOEF)
    if mode == "f32_split":
        yb_s = _shard(np.ascontiguousarray(
            np.concatenate([y, B_tl], axis=1)), 2)
        return [
            {"mo": mo_s[i], "a": a_s[i], "yb": yb_s[i], "vb": vb}
            for i in range(N_CORES)
        ]
    y_s = _shard(y, 1)
    b_s = _shard(B_tl, 1)
    return [
        {"mo": mo_s[i], "a": a_s[i], "y": y_s[i], "b": b_s[i], "vb": vb}
        for i in range(N_CORES)
    ]


def kernel(model_output, y, A, B_tl, beta_TL):
    nc = _build_nc()
    in_maps = _prepare_in_maps(model_output, y, A, B_tl, beta_TL)
    res = None
    for attempt in range(3):
        try:
            res = run_bass_kernel_spmd(nc, in_maps, list(range(N_CORES)))
            break
        except Exception:
            # transient device wedge (NRT_EXEC_UNIT_UNRECOVERABLE etc.)
            # recovers on a fresh attempt; re-raise only if persistent
            if attempt == 2:
                raise
            import time
            time.sleep(5)
    total = 0.0
    for r in res.results:
        total += float(r["out"].astype(np.float64).sum())
    return np.asarray(total / N_TOTAL, dtype=np.float32)

